# revision 9
# baseline (speedup 1.0000x reference)
"""Trainium2 Bass kernel for the Galaxy Zoo CNN (segment_reduce problem).

Data-parallel over 8 NeuronCores: 128 images per core.

Per-core dataflow (all matmuls bf16, PSUM fp32):
  conv1: col-pack S=4, full ky in K: K=(ky,g,ci)=5*8*3=120 (+1 ones/bias row),
         M=(rep,Dlt,co)=2*4*16=128, N=(oh,blk)=56*14=784 (chunks 504+280)
  conv2: S=4: K=(g,ci)=8*16=128 per ky (5 PSUM-accum passes),
         M=(blkpos(dlt),co)=128 with dlt order [0,2,1,3] so 2x2 maxpool's
         column pairs are partition halves; N=52*13=676 (chunks 494+182)
  conv3: S=2: K=(g,ci)=4*32=128 per ky (3 passes), M=(dlt,co)=128, N=24*12=288
  conv4: S=1: K chunks (g in {0,1})x64=128 and (g=2)x64, 3 ky passes,
         M=co=128, N=100
  pools: colmax = (PSUM_half0 + bias) max SBUF-evacuated-half1 (one
         scalar_tensor_tensor), rowmax+relu fused into the im2col writes of
         the next layer's Z (stt max/max).
  fc1/fc2: pixel-major K chunks; fc3 computed data-as-lhsT so the output
         lands as [img, 37] directly; sigmoid on ACT; the faithful
         sequential segment-normalize + dependency-multiply run on DVE.
"""
import numpy as np
import ml_dtypes

BF16 = ml_dtypes.bfloat16
BN_EPS = 1e-5
Q_A = {1: (0, 3), 2: (3, 5), 3: (5, 7), 4: (7, 9), 5: (9, 13), 6: (13, 15),
       7: (15, 18), 8: (18, 25), 9: (25, 28), 10: (28, 31), 11: (31, 37)}
DEP_ORDER = [(2, 1), (3, 4), (4, 4), (5, 4), (7, 0), (8, 13), (9, 3), (10, 7), (11, 7)]
BLKPOS = [0, 2, 1, 3]
N_CORES = 8
B_CORE = 128
GRP = 2  # images per pipeline group

# ----------------------------------------------------------------- host prep

def _fold_bn(inp):
    out, a, c = {}, {}, {}
    for L in (1, 2, 3, 4):
        s, b, m, v = (inp[f'bn{L}_s'], inp[f'bn{L}_b'], inp[f'bn{L}_m'], inp[f'bn{L}_v'])
        a[L] = s / np.sqrt(v + BN_EPS)
        c[L] = b - m * a[L]
    out['W1'], out['b1'] = inp['conv1_w'], inp['conv1_b']
    for L in (2, 3, 4):
        W = inp[f'conv{L}_w']
        aL, cL = a[L - 1], c[L - 1]
        out[f'W{L}'] = W * aL[None, :, None, None]
        out[f'b{L}'] = inp[f'conv{L}_b'] + (W.sum(axis=(2, 3)) * cL[None, :]).sum(axis=1)
    a4c = np.repeat(a[4], 25)
    c4c = np.repeat(c[4], 25)
    out['fw1'] = inp['fc1_w'] * a4c[None, :]
    out['fb1'] = inp['fc1_b'] + inp['fc1_w'] @ c4c
    out['fw2'], out['fb2'] = inp['fc2_w'], inp['fc2_b']
    out['fw3'], out['fb3'] = inp['fc3_w'], inp['fc3_b']
    return out


def _build_z1(x):
    B = x.shape[0]
    Z = np.empty((B, 5, 8, 3, 56, 14), np.float32)
    for d in range(5):
        for g in range(8):
            Z[:, d, g, :, :, :] = x[:, :, d:d + 56, g:g + 53:4][:, :, :, :14]
    Z = Z.reshape(B, 120, 784)
    return np.concatenate([Z, np.ones((B, 1, 784), np.float32)], axis=1)


def _conv1_lhsT(W1, b1):
    L = np.zeros((121, 128), np.float32)
    for d in range(5):
        for g in range(8):
            for ci in range(3):
                r = (d * 8 + g) * 3 + ci
                for D in range(4):
                    kx = g - D
                    if 0 <= kx <= 4:
                        L[r, D * 16:(D + 1) * 16] = W1[:, ci, d, kx]
    L[120, 0:64] = np.tile(b1, 4)
    L[:, 64:128] = L[:, 0:64]  # replicated half for partition-aligned T1 copies
    return L


def _conv2_lhsT(W2):
    L = np.zeros((128, 5 * 128), np.float32)
    for ky in range(5):
        for g in range(8):
            for ci in range(16):
                r = g * 16 + ci
                for d in range(4):
                    kx = g - d
                    if 0 <= kx <= 4:
                        c0 = ky * 128 + BLKPOS[d] * 32
                        L[r, c0:c0 + 32] = W2[:, ci, ky, kx]
    return L


def _conv3_lhsT(W3):
    L = np.zeros((128, 3 * 128), np.float32)
    for ky in range(3):
        for g in range(4):
            for ci in range(32):
                r = g * 32 + ci
                for d in range(2):
                    kx = g - d
                    if 0 <= kx <= 2:
                        c0 = ky * 128 + d * 64
                        L[r, c0:c0 + 64] = W3[:, ci, ky, kx]
    return L


def _conv4_lhsT(W4):
    A = np.zeros((128, 3 * 128), np.float32)
    Bm = np.zeros((64, 3 * 128), np.float32)
    for ky in range(3):
        for ci in range(64):
            A[ci, ky * 128:(ky + 1) * 128] = W4[:, ci, ky, 0]
            A[64 + ci, ky * 128:(ky + 1) * 128] = W4[:, ci, ky, 1]
            Bm[ci, ky * 128:(ky + 1) * 128] = W4[:, ci, ky, 2]
    return A, Bm


def _prep_host(inputs):
    F = _fold_bn({k: np.asarray(v, np.float32) for k, v in inputs.items()})
    dev = {}
    dev['l1'] = _conv1_lhsT(F['W1'], F['b1']).astype(BF16)
    dev['l2'] = _conv2_lhsT(F['W2']).astype(BF16)
    dev['l3'] = _conv3_lhsT(F['W3']).astype(BF16)
    l4a, l4b = _conv4_lhsT(F['W4'])
    dev['l4a'] = l4a.astype(BF16)
    dev['l4b'] = l4b.astype(BF16)
    # fc1 lhsT: [co, (j, m, n)]
    dev['fw1'] = (F['fw1'].reshape(8, 128, 128, 25).transpose(2, 3, 0, 1)
                  .reshape(128, 25600).astype(BF16))
    dev['fw2'] = (F['fw2'].reshape(4, 128, 8, 128).transpose(3, 2, 0, 1)
                  .reshape(128, 4096).astype(BF16))
    dev['fw3'] = (F['fw3'].reshape(37, 4, 128).transpose(2, 1, 0)
                  .reshape(128, 148).astype(BF16))
    dev['bias2'] = np.tile(F['b2'], 4).astype(np.float32).reshape(128, 1)
    dev['bias3'] = np.tile(F['b3'], 2).astype(np.float32).reshape(128, 1)
    dev['bias4'] = F['b4'].astype(np.float32).reshape(128, 1)
    dev['fb1'] = F['fb1'].astype(np.float32).reshape(8, 128).T.copy()   # [128, 8]
    dev['fb2'] = F['fb2'].astype(np.float32).reshape(4, 128).T.copy()   # [128, 4]
    dev['fb3'] = F['fb3'].astype(BF16).reshape(1, 37)
    z1 = _build_z1(np.asarray(inputs['x'], np.float32)).astype(BF16)    # [1024, 121, 784]
    return dev, z1

# --------------------------------------------------------------- bass module

_CACHED = {}


def _build_module():
    import concourse.bacc as bacc
    import concourse.mybir as mybir
    from concourse.tile import TileContext

    dt = mybir.dt
    Alu = mybir.AluOpType
    Act = mybir.ActivationFunctionType

    nc = bacc.Bacc(None, target_bir_lowering=False)

    z1d = nc.dram_tensor("z1", [B_CORE, 121, 784], dt.bfloat16, kind="ExternalInput")
    l1d = nc.dram_tensor("l1", [121, 128], dt.bfloat16, kind="ExternalInput")
    l2d = nc.dram_tensor("l2", [128, 640], dt.bfloat16, kind="ExternalInput")
    l3d = nc.dram_tensor("l3", [128, 384], dt.bfloat16, kind="ExternalInput")
    l4ad = nc.dram_tensor("l4a", [128, 384], dt.bfloat16, kind="ExternalInput")
    l4bd = nc.dram_tensor("l4b", [64, 384], dt.bfloat16, kind="ExternalInput")
    fw1d = nc.dram_tensor("fw1", [128, 25600], dt.bfloat16, kind="ExternalInput")
    fw2d = nc.dram_tensor("fw2", [128, 4096], dt.bfloat16, kind="ExternalInput")
    fw3d = nc.dram_tensor("fw3", [128, 148], dt.bfloat16, kind="ExternalInput")
    bias2d = nc.dram_tensor("bias2", [128, 1], dt.float32, kind="ExternalInput")
    bias3d = nc.dram_tensor("bias3", [128, 1], dt.float32, kind="ExternalInput")
    bias4d = nc.dram_tensor("bias4", [128, 1], dt.float32, kind="ExternalInput")
    fb1d = nc.dram_tensor("fb1", [128, 8], dt.float32, kind="ExternalInput")
    fb2d = nc.dram_tensor("fb2", [128, 4], dt.float32, kind="ExternalInput")
    fb3d = nc.dram_tensor("fb3", [1, 37], dt.bfloat16, kind="ExternalInput")
    outd = nc.dram_tensor("out", [B_CORE, 37], dt.float32, kind="ExternalOutput")

    with TileContext(nc) as tc:
        with tc.tile_pool(name="wpool", bufs=1) as wp, \
             tc.tile_pool(name="big", bufs=1) as bigp, \
             tc.tile_pool(name="io", bufs=3) as iop, \
             tc.tile_pool(name="work", bufs=2) as wk, \
             tc.tile_pool(name="psum", bufs=1, space="PSUM") as pp:

            def load(dram, shape, tag, dtype=dt.bfloat16, pool=wp):
                t = pool.tile(shape, dtype, tag=tag)
                nc.sync.dma_start(out=t[:], in_=dram[:])
                return t

            l1t = load(l1d, [121, 128], "l1t")
            l2t = load(l2d, [128, 640], "l2t")
            l3t = load(l3d, [128, 384], "l3t")
            l4at = load(l4ad, [128, 384], "l4at")
            l4bt = load(l4bd, [64, 384], "l4bt")
            fw2t = load(fw2d, [128, 4096], "fw2t")
            fw3t = load(fw3d, [128, 148], "fw3t")
            b2t = load(bias2d, [128, 1], "b2t", dt.float32)
            b3t = load(bias3d, [128, 1], "b3t", dt.float32)
            b4t = load(bias4d, [128, 1], "b4t", dt.float32)
            fb1t = load(fb1d, [128, 8], "fb1t", dt.float32)
            fb2t = load(fb2d, [128, 4], "fb2t", dt.float32)
            fb3t = load(fb3d, [1, 37], "fb3t")
            fw1t = load(fw1d, [128, 25600], "fw1t", pool=bigp)

            p4t = bigp.tile([128, 3200], dt.bfloat16)     # [co, (j, img)]
            h1t = bigp.tile([128, 1024], dt.bfloat16)     # [n, (m, img)]
            h2t = bigp.tile([128, 512], dt.bfloat16)
            onest = wp.tile([1, 128], dt.bfloat16)
            nc.vector.memset(onest[:], 1.0)

            for g in range(B_CORE // GRP):
                i0 = g * GRP
                z1t = iop.tile([121, GRP, 784], dt.bfloat16, tag="z1t")
                nc.sync.dma_start(
                    out=z1t[:], in_=z1d[i0:i0 + GRP].rearrange("i p n -> p i n"))

                # ---------------- conv1 (per-image PSUM, 2 chunks each)
                z2t = wk.tile([128, GRP, 56, 13], dt.bfloat16, tag="z2t")
                for i in range(GRP):
                    c1 = pp.tile([128, 1024], dt.float32, tag="c1")
                    for off, n in ((0, 504), (512, 280)):
                        zoff = 0 if off == 0 else 504
                        nc.tensor.matmul(
                            c1[:, off:off + n], l1t[:],
                            z1t[:, i, zoff:zoff + n], start=True, stop=True)
                    # T1 -> Z2  [128=(g2,ci), (img, h 56, b2 13)]
                    for off, n_oh, oh0 in ((0, 36, 0), (512, 20, 36)):
                        vA = c1[0:64, off:off + n_oh * 14].rearrange(
                            "p (h b) -> p h b", b=14)
                        vB = c1[64:128, off:off + n_oh * 14].rearrange(
                            "p (h b) -> p h b", b=14)
                        nc.vector.tensor_scalar(
                            out=z2t[0:64, i, oh0:oh0 + n_oh, :], in0=vA[:, :, 0:13],
                            scalar1=0.0, scalar2=None, op0=Alu.max)
                        nc.scalar.activation(
                            z2t[64:128, i, oh0:oh0 + n_oh, :], vB[:, :, 1:14], Act.Relu)

                # ---------------- conv2 (5 ky accum passes)
                c2 = pp.tile([128, GRP, 1024], dt.float32, tag="c2")
                z2f = z2t[:, :, :, :].rearrange("p i h b -> p i (h b)")
                for i in range(GRP):
                    for off, n, s0 in ((0, 494, 0), (512, 182, 494)):
                        for ky in range(5):
                            nc.tensor.matmul(
                                c2[:, i, off:off + n],
                                l2t[:, ky * 128:(ky + 1) * 128],
                                z2f[:, i, ky * 13 + s0: ky * 13 + s0 + n],
                                start=(ky == 0), stop=(ky == 4))
                # T2: evac half1 (+bias), colmax (+bias), rowmax+relu -> Z3
                e2 = wk.tile([64, GRP, 676], dt.float32, tag="e2")
                cm2 = wk.tile([64, GRP, 52, 13], dt.bfloat16, tag="cm2")
                cm2f = cm2[:, :, :, :].rearrange("p i h b -> p i (h b)")
                for off, n, s0 in ((0, 494, 0), (512, 182, 494)):
                    nc.scalar.activation(
                        e2[:, :, s0:s0 + n], c2[64:128, :, off:off + n],
                        Act.Identity, bias=b2t[64:128, 0:1])
                    nc.vector.scalar_tensor_tensor(
                        out=cm2f[:, :, s0:s0 + n], in0=c2[0:64, :, off:off + n],
                        scalar=b2t[0:64, 0:1], in1=e2[:, :, s0:s0 + n],
                        op0=Alu.add, op1=Alu.max)
                z3t = wk.tile([128, GRP, 26, 12], dt.bfloat16, tag="z3t")
                z3m = z3t[:, :, :, :].rearrange("p i h b -> p (i h) b")
                cm2r = cm2[:, :, :, :].rearrange("p i h b -> p (i h b)").rearrange(
                    "p (a b) -> p a b", b=26)
                for lo, m0, eng in ((True, 0, nc.vector), (False, 1, nc.vector)):
                    dst = z3m[0:64] if lo else z3m[64:128]
                    eng.scalar_tensor_tensor(
                        out=dst,
                        in0=cm2r[:, :, m0:m0 + 12],
                        scalar=0.0,
                        in1=cm2r[:, :, 13 + m0:13 + m0 + 12],
                        op0=Alu.max, op1=Alu.max)

                # ---------------- conv3 (3 ky)
                c3 = pp.tile([128, GRP, 512], dt.float32, tag="csm")
                z3f = z3t[:, :, :, :].rearrange("p i h b -> p i (h b)")
                for i in range(GRP):
                    for ky in range(3):
                        nc.tensor.matmul(
                            c3[:, i, 0:288], l3t[:, ky * 128:(ky + 1) * 128],
                            z3f[:, i, ky * 12: ky * 12 + 288],
                            start=(ky == 0), stop=(ky == 2))
                e3 = wk.tile([64, GRP, 288], dt.float32, tag="e3")
                cm3 = wk.tile([64, GRP, 24, 12], dt.bfloat16, tag="cm3")
                nc.scalar.activation(e3[:, :, :], c3[64:128, :, 0:288],
                                     Act.Identity, bias=b3t[64:128, 0:1])
                nc.vector.scalar_tensor_tensor(
                    out=cm3[:, :, :, :].rearrange("p i h b -> p i (h b)"),
                    in0=c3[0:64, :, 0:288], scalar=b3t[0:64, 0:1],
                    in1=e3[:, :, :], op0=Alu.add, op1=Alu.max)
                z4a = wk.tile([128, GRP, 12, 10], dt.bfloat16, tag="z4a")
                z4b = wk.tile([64, GRP, 12, 10], dt.bfloat16, tag="z4b")
                z4am = z4a[:, :, :, :].rearrange("p i h b -> p (i h) b")
                z4bm = z4b[:, :, :, :].rearrange("p i h b -> p (i h) b")
                cm3r = cm3[:, :, :, :].rearrange("p i h b -> p (i h b)").rearrange(
                    "p (a b) -> p a b", b=24)
                for dst, m0, eng in ((z4am[0:64], 0, nc.vector),
                                     (z4am[64:128], 1, nc.vector),
                                     (z4bm[0:64], 2, nc.vector)):
                    eng.scalar_tensor_tensor(
                        out=dst,
                        in0=cm3r[:, :, m0:m0 + 10], scalar=0.0,
                        in1=cm3r[:, :, 12 + m0:12 + m0 + 10],
                        op0=Alu.max, op1=Alu.max)

                # ---------------- conv4 (3 ky x 2 K-chunks)
                c4 = pp.tile([128, GRP, 512], dt.float32, tag="csm")
                z4af = z4a[:, :, :, :].rearrange("p i h b -> p i (h b)")
                z4bf = z4b[:, :, :, :].rearrange("p i h b -> p i (h b)")
                for i in range(GRP):
                    for ky in range(3):
                        nc.tensor.matmul(
                            c4[:, i, 0:100], l4at[:, ky * 128:(ky + 1) * 128],
                            z4af[:, i, ky * 10: ky * 10 + 100],
                            start=(ky == 0), stop=False)
                        nc.tensor.matmul(
                            c4[:, i, 0:100], l4bt[:, ky * 128:(ky + 1) * 128],
                            z4bf[:, i, ky * 10: ky * 10 + 100],
                            start=False, stop=(ky == 2))
                # T4: evac+bias, colmax, rowmax+relu -> P4 strided
                e4 = wk.tile([128, GRP, 10, 10], dt.float32, tag="e4")
                cm4 = wk.tile([128, GRP, 10, 5], dt.float32, tag="cm4")
                nc.scalar.activation(
                    e4[:, :, :, :].rearrange("p i h b -> p i (h b)"),
                    c4[:, :, 0:100], Act.Identity, bias=b4t[:, 0:1])
                e4r = e4[:, :, :, :].rearrange("p i h b -> p (i h) b")
                nc.vector.tensor_tensor(
                    out=cm4[:, :, :, :].rearrange("p i h b -> p (i h) b"),
                    in0=e4r[:, :, 0:10:2], in1=e4r[:, :, 1:10:2], op=Alu.max)
                # p4t layout: col = img*25 + (h'*5+w')
                cm4r = cm4[:, :, :, :].rearrange("p i h b -> p (i h b)").rearrange(
                    "p (a b) -> p a b", b=10)
                p4v = p4t[:, i0 * 25:(i0 + GRP) * 25].rearrange(
                    "p (a w) -> p a w", w=5)
                nc.vector.scalar_tensor_tensor(
                    out=p4v, in0=cm4r[:, :, 0:5], scalar=0.0,
                    in1=cm4r[:, :, 5:10], op0=Alu.max, op1=Alu.max)

            # ---------------- FC layers
            for m in range(8):
                f1 = pp.tile([128, 128], dt.float32, tag="c1")
                for j in range(25):
                    nc.tensor.matmul(
                        f1[:], fw1t[:, (j * 8 + m) * 128:(j * 8 + m + 1) * 128],
                        p4t[:, :].rearrange("p (i j) -> p i j", j=25)[:, :, j],
                        start=(j == 0), stop=(j == 24))
                nc.scalar.activation(h1t[:, m * 128:(m + 1) * 128], f1[:],
                                     Act.Relu, bias=fb1t[:, m:m + 1])
            for m in range(4):
                f2 = pp.tile([128, 128], dt.float32, tag="c1")
                for k in range(8):
                    nc.tensor.matmul(
                        f2[:], fw2t[:, (k * 4 + m) * 128:(k * 4 + m + 1) * 128],
                        h1t[:, k * 128:(k + 1) * 128],
                        start=(k == 0), stop=(k == 7))
                nc.scalar.activation(h2t[:, m * 128:(m + 1) * 128], f2[:],
                                     Act.Relu, bias=fb2t[:, m:m + 1])
            f3 = pp.tile([128, 37], dt.float32, tag="c1")
            for k in range(4):
                nc.tensor.matmul(f3[:], h2t[:, k * 128:(k + 1) * 128],
                                 fw3t[:, k * 37:(k + 1) * 37],
                                 start=(k == 0), stop=False)
            nc.tensor.matmul(f3[:], onest[:], fb3t[:], start=False, stop=True)
            xsg = wp.tile([128, 37], dt.float32)
            nc.scalar.activation(xsg[:], f3[:], Act.Sigmoid)

            # ---------------- faithful sequential segment normalize + multiply
            sumt = wp.tile([128, 1], dt.float32)
            rect = wp.tile([128, 1], dt.float32)
            for qi in range(1, 12):
                s, e = Q_A[qi]
                for j in range(s, e):
                    nc.vector.reduce_sum(sumt[:], xsg[:, s:e],
                                         axis=mybir.AxisListType.X)
                    nc.vector.reciprocal(rect[:], sumt[:])
                    nc.vector.tensor_scalar(
                        out=xsg[:, j:j + 1], in0=xsg[:, j:j + 1],
                        scalar1=rect[:, 0:1], scalar2=None, op0=Alu.mult)
            for qi, v in DEP_ORDER:
                s, e = Q_A[qi]
                nc.vector.tensor_scalar(
                    out=xsg[:, s:e], in0=xsg[:, s:e],
                    scalar1=xsg[:, v:v + 1], scalar2=None, op0=Alu.mult)
            nc.sync.dma_start(out=outd[:], in_=xsg[:])

    nc.compile()
    return nc


def kernel(**inputs):
    from concourse.bass_utils import run_bass_kernel_spmd

    if 'nc' not in _CACHED:
        _CACHED['nc'] = _build_module()
    nc = _CACHED['nc']
    dev, z1 = _prep_host(inputs)
    in_maps = []
    for c in range(N_CORES):
        m = dict(dev)
        m['z1'] = np.ascontiguousarray(z1[c * B_CORE:(c + 1) * B_CORE])
        in_maps.append(m)
    res = run_bass_kernel_spmd(nc, in_maps, core_ids=list(range(N_CORES)))
    out = np.concatenate([res.results[c]['out'] for c in range(N_CORES)], axis=0)
    return out.astype(np.float32)
